# revision 83
# baseline (speedup 1.0000x reference)
"""Trainium2 Bass kernel for nn_MGEVelocityIntr.

Reference semantics: build a 4096-point log-radius grid, evaluate the MGE
circular-velocity curve v(R) on it (BH point mass + 2048-term Gauss-Legendre
quadrature of the MGE integral), then linearly interpolate every pixel of the
4096x4096 R_map onto that curve.

TRN2 has no per-lane gather, so instead of table interpolation we evaluate a
smooth surrogate of the whole curve per pixel.  In log-log space the curve
wt(m) = ln v(e^m) is gentle (slope in [-0.6, 0.3]); it is fitted on the host
(from the small MGE parameter vectors only) with

    wt(m) ~= c0 + c1*m + sum_k a_k tanh(s_k m + b_k)        [NT terms, ACT]
           + sum_j max/min(s1_j m, s2_j)                     [NH hinges, DVE]

and the pixel pipeline is   v = exp(wt(ln x)).

Engine mapping (the whole point of this design):
  DVE+GPS: m = ln x computed WITHOUT the ACT engine via the bf16 bit
          pattern ("fast log"): m = ln2/128*i - 127*ln2 + a*s*(1-s), where
          i is the int16 view of the bf16 input and s = (i & 127)/128 the
          mantissa fraction (parabola corrects the log1p sawtooth, <=5.3e-3)
  DVE   : NH x tensor_scalar f16 hinges (4x perf mode, ~550ns/1k chunk)
  GPS   : fast-log products + one hinge pair-sum (otherwise idle engine)
  ACT   : NT x Tanh + final Exp only (4 ops/pixel, the bottleneck at
          ~61us/core); a single exp_and_others table set, loaded once
  PE    : accumulates ALL terms into PSUM via diagonal f16 matmuls (fp32
          accumulate), staged per tanh so the tail stays short; the linear
          c1*m term is just one more matmul on the m tile
  Exp reads the PSUM accumulator directly and writes v/16 as f16 (the host
  multiplies back); input is downcast to bf16 on the host, halving DMA.

Total surrogate error vs the reference curve (incl all f16/bf16 rounding):
~8e-3 max relative, vs the 2e-2 gate.  Coefficients (cf vector + diagonal
weight blocks) are runtime inputs; only the max/min hinge op pattern is
baked into the NEFF (cached per pattern).

Sharding: pure data-parallel, 512 R_map rows per core across 8 cores.
"""

import numpy as np

N_CORES = 8
ROWS = 4096
COLS = 4096
ROWS_PER_CORE = ROWS // N_CORES          # 512
FREE = ROWS_PER_CORE * COLS // 128       # 16384 free elems per partition
CH = 4096                                # big chunk (DMA/Ln/tanh/hinges)
NCHUNK = FREE // CH                      # 4
SC = 1024                                # PSUM sub-chunk (matmul/Exp/DMA-out)
NSUB = CH // SC                          # 4
MM = 512                                 # matmul moving-dim tile (PSUM bank)

NT = 3                                   # tanh terms (ACT)
NH = 5                                   # hinge terms (DVE tensor_scalar)
NHT = 3                                  # hinge tiles after pair-summing
NTERMS = 1 + NHT + NT                    # linear + hinge tiles + tanh
NCOEF = 4 + 2 * NT + 2 * NH
OUT_SCALE = 16.0                         # f16 output is v/OUT_SCALE

# fast-log: ln x = ln2/128 * int16(bf16 bits) - 127*ln2 + c(s), s = mant/128,
# approximated with c(s) ~= FL_ALPHA * s * (1 - s)  (minimax over the 128
# mantissa values, residual <= 5.3e-3 in ln x)
LN2 = float(np.log(2.0))
FL_ALPHA = 0.24021
FL_C1 = float(FL_ALPHA ** 0.5 / 128.0)   # mask-path scale: sqrt(alpha)/128
FL_CH = 1024                             # fast-log pipeline granularity
NFL = FREE // FL_CH

SOFT = 0.01
G = 0.004301
QUAD_POINTS = 128


# ---------------------------------------------------------------------------
# Host-side model + fit (uses only the small MGE parameter inputs)
# ---------------------------------------------------------------------------

def _exact_curve_params(surf, sigma, qintr, M_to_L, inc, m_bh):
    """Exact (float64) A,B such that vc2_mge(x) = mge_coef * sum A*exp(-B*z),
    z=(x/scale)^2, mirroring the reference's quadrature."""
    x0, w0 = np.polynomial.legendre.leggauss(QUAD_POINTS)
    x0 = x0.astype(np.float32).astype(np.float64)
    w0 = w0.astype(np.float32).astype(np.float64)
    surf = surf.astype(np.float64)
    sigma = sigma.astype(np.float64)
    qintr = qintr.astype(np.float64)
    inc = float(inc)
    sqrt_2pi = np.sqrt(2.0 * np.pi)
    qobs = np.sqrt(qintr**2 * np.sin(inc) ** 2 + np.cos(inc) ** 2)
    md = surf * float(M_to_L) * qobs / (qintr * sigma * sqrt_2pi)
    scale = np.quantile(sigma, 0.5)
    ssc = sigma / scale
    mds = np.quantile(ssc, 0.5)
    mxs = ssc.max()
    lo = np.arcsinh(np.log(1e-7 * mds) * 2.0 / np.pi)
    hi = np.arcsinh(np.log(1000.0 * mxs) * 2.0 / np.pi)
    half = 0.5 * (hi - lo)
    mid = 0.5 * (hi + lo)
    t1 = half * x0 + mid
    w1 = half * w0
    u1 = np.exp(np.pi / 2.0 * np.sinh(t1))
    du1 = np.pi / 2.0 * np.cosh(t1) * u1
    one = 1.0 + u1
    B = 0.5 / (ssc[None, :] ** 2 * one[:, None])                        # [Q,C]
    A = (
        qintr[None, :] * md[None, :]
        / (one[:, None] ** 2 * np.sqrt(qintr[None, :] ** 2 + u1[:, None]))
        * (du1 * w1)[:, None]
    )
    mge_coef = 2.0 * np.pi * G * scale**2
    bh_coef = G * 10.0 ** float(m_bh) / scale
    return A.ravel(), B.ravel(), float(scale), mge_coef, bh_coef


def _curve_wt(xs, A, B, scale, mge_coef, bh_coef):
    z = (xs / scale) ** 2
    ssc2 = (SOFT / scale) ** 2
    I = (A[None, :] * np.exp(-np.outer(z, B))).sum(1)
    vc2 = mge_coef * I + bh_coef * (z + ssc2) ** (-1.5)
    # reference returns R_sc * sqrt(vc2) with R_sc = R / scale
    return np.log(xs / scale) + 0.5 * np.log(vc2)


def _fit_surrogate(A, B, scale, mge_coef, bh_coef, x_lo, x_hi, seed=0):
    """Fit wt(u), u = ln x - mid, with NT tanh + NH oriented hinges + linear.

    Returns (mid, c0, c1, tanh_s, tanh_b, tanh_a, hinge_s1, hinge_s2,
    hinge_sign, fit_report)."""
    import scipy.optimize as so

    xs = np.exp(np.linspace(np.log(x_lo), np.log(x_hi), 6000))
    wt = _curve_wt(xs, A, B, scale, mge_coef, bh_coef)
    mt = np.log(xs)
    mid = 0.5 * (mt.min() + mt.max())
    u = mt - mid
    ulo, uhi = u.min(), u.max()

    # curvature density -> adaptive knot seeds (works for any MGE params)
    h = u[1] - u[0]
    w2 = np.gradient(np.gradient(wt, h), h)
    dens = np.sqrt(np.abs(w2)) + 1e-4
    cdf = np.cumsum(dens)
    cdf /= cdf[-1]

    def quantile_knots(n):
        qs = (np.arange(n) + 0.5) / n
        return np.interp(qs, cdf, u)

    def design(p):
        ts_, bs_ = p[:NT], p[NT : 2 * NT]
        knots = p[2 * NT :]
        cols = [np.ones_like(u), u]
        for k in range(NT):
            cols.append(np.tanh(ts_[k] * u + bs_[k]))
        for j in range(NH):
            t = knots[j]
            if t >= 0:
                cols.append(np.maximum(u - t, 0.0))
            else:
                cols.append(np.maximum(t - u, 0.0))
        return np.column_stack(cols)

    def solve(p):
        Phi = design(p)
        coef, *_ = np.linalg.lstsq(Phi, wt, rcond=None)
        return coef, Phi @ coef - wt

    def resid(p):
        return solve(p)[1]

    rng = np.random.RandomState(seed)
    best = None
    tanh_seed_centers = quantile_knots(NT)
    for trial in range(10):
        tc = tanh_seed_centers + rng.randn(NT) * 0.4
        s0 = rng.uniform(0.8, 3.0, NT)
        b0 = -s0 * tc
        kn = np.sort(quantile_knots(NH) + rng.randn(NH) * 0.3)
        kn = np.clip(kn, ulo + 0.2, uhi - 0.2)
        p0 = np.concatenate([s0, b0, kn])
        try:
            res = so.least_squares(
                resid, p0, method="trf", max_nfev=250, x_scale="jac"
            )
        except Exception:
            continue
        e = float(np.abs(resid(res.x)).max())
        if best is None or e < best[1]:
            best = (res.x, e)
    p, fit_err = best
    coef, _ = solve(p)

    tanh_s = p[:NT].copy()
    tanh_b = p[NT : 2 * NT].copy()
    knots = p[2 * NT :].copy()
    c0, c1 = coef[0], coef[1]
    tanh_a = coef[2 : 2 + NT].copy()
    hinge_a = coef[2 + NT :].copy()

    # snap PE-side coefficients (tanh_a and c1 diag values; hinge signs are
    # exact) to f16 and re-solve the remaining linear coefficients.
    tanh_a16 = np.float16(tanh_a).astype(np.float64)
    Phi = design(p)
    target2 = wt - Phi[:, 2 : 2 + NT] @ tanh_a16
    Phi2 = np.column_stack([Phi[:, :2], Phi[:, 2 + NT :]])
    coef2, *_ = np.linalg.lstsq(Phi2, target2, rcond=None)
    c1 = float(np.float16(coef2[1]))
    target3 = target2 - c1 * u
    Phi3 = np.column_stack([Phi[:, :1], Phi[:, 2 + NT :]])
    coef3, *_ = np.linalg.lstsq(Phi3, target3, rcond=None)
    c0 = coef3[0]
    hinge_a = coef3[1:].copy()

    # device hinge encoding: SIGNED tile  phi_j = op1(s1_j*u, s2_j)  with
    # op1 = max for a_j > 0 and min for a_j < 0 (sign lives in the ALU op, a
    # build-time choice), s1 = +-|a| by orientation, s2 = s1*t always:
    #   a>0,R: max(a u, a t)      = a(u-t)+ + s2
    #   a>0,L: max(-a u, -a t)    = a(t-u)+ + s2
    #   a<0,R: min(a u, a t)      = a(u-t)+ + s2
    #   a<0,L: min(-a u, -a t)    = a(t-u)+ + s2
    hinge_s1 = np.empty(NH)
    hinge_s2 = np.empty(NH)
    hinge_ismax = np.empty(NH, dtype=bool)
    const_extra = 0.0
    for j in range(NH):
        a, t = hinge_a[j], knots[j]
        if abs(a) < 1e-12:
            a = 1e-12
        right = t >= 0
        s1 = a if right else -a
        s2 = s1 * t
        hinge_s1[j] = s1
        hinge_s2[j] = s2
        hinge_ismax[j] = a > 0
        const_extra += s2
    # each tile contributes +s2_j of constant; remove it from the Exp bias
    c0_total = c0 - const_extra

    report = dict(fit_err=fit_err, mid=mid, ulo=ulo, uhi=uhi)
    return (
        mid, c0_total, c1, tanh_s, tanh_b, tanh_a16,
        hinge_s1, hinge_s2, hinge_ismax, report,
    )


def _emulate_device(x, exp_bias, c1, ts_, tbp_, ta_, hs1, hs2p, hismax):
    """Host replica of the device pipeline including bf16/f16 rounding and
    the bit-level fast-log.  All parameters are the DEVICE-level values
    (uncentered m = ln x; exp_bias excludes the OUT_SCALE shift)."""
    f16 = np.float16
    xb32 = _to_bf16(np.float32(x))
    i = (xb32.view(np.uint32) >> 16).astype(np.int64)   # bf16 bit pattern
    mraw = f16(i * (LN2 / 128.0) - 127.0 * LN2).astype(np.float64)
    t1 = f16((i & 127) * FL_C1).astype(np.float64)
    t2 = f16(128.0 * FL_C1 - t1).astype(np.float64)
    g = f16(t1 * t2).astype(np.float64)
    m = f16(mraw + g).astype(np.float64)
    acc = np.zeros_like(m)
    acc += f16(c1).astype(np.float64) * m          # linear term via PE weight
    phis = []
    for j in range(NH):
        raw = hs1[j] * m
        phi = np.maximum(raw, hs2p[j]) if hismax[j] else np.minimum(raw, hs2p[j])
        phis.append(f16(phi).astype(np.float64))
    # hinge pairs are pre-summed (DVE/GPS) in f16 before PE accumulation
    acc += f16(phis[0] + phis[1]).astype(np.float64)
    acc += f16(phis[2] + phis[3]).astype(np.float64)
    acc += phis[4]
    for k in range(NT):
        phi = f16(np.tanh(ts_[k] * m + tbp_[k])).astype(np.float64)
        acc += ta_[k] * phi
    # device Exp writes f16 scaled by 1/OUT_SCALE; host multiplies back
    v = np.exp(acc + exp_bias - np.log(OUT_SCALE)).astype(np.float16)
    return v.astype(np.float64) * OUT_SCALE


def _to_bf16(a_f32):
    """Round-to-nearest-even f32 -> bf16, returned as f32 values."""
    u = a_f32.view(np.uint32)
    rounded = (u + 0x7FFF + ((u >> 16) & 1)) & 0xFFFF0000
    return rounded.view(np.float32)


_FIT_CACHE = {}


def _coef_arrays(surf, sigma, qintr, M_to_L, inc, m_bh, r_max):
    key = (surf.tobytes(), sigma.tobytes(), qintr.tobytes(), M_to_L, inc, m_bh,
           round(float(r_max), 6))
    if key in _FIT_CACHE:
        return _FIT_CACHE[key]
    A, B, scale, mge_coef, bh_coef = _exact_curve_params(
        surf, sigma, qintr, M_to_L, inc, m_bh
    )
    x_lo = 0.99 * SOFT
    x_hi = 1.03 * float(r_max)
    (mid, c0, c1, ts_, tb_, ta_, hs1, hs2, hismax, rep) = _fit_surrogate(
        A, B, scale, mge_coef, bh_coef, x_lo, x_hi
    )

    # translate the centered fit (u = ln x - mid) to the UNCENTERED device
    # variable m = ln x:  tanh bias b' = b - s*mid;  hinge s2' = s1*(t+mid)
    # (each hinge tile then carries +s1*mid of extra constant);  linear term
    # contributes +c1*mid; all constants move into the Exp bias.
    tbp = tb_ - ts_ * mid
    hs2p = hs2 + hs1 * mid
    exp_bias = c0 - mid * (c1 + float(np.sum(hs1)))

    # accuracy audit on a dense grid (device emulation incl f16 rounding)
    xs = np.exp(np.linspace(np.log(x_lo), np.log(x_hi), 20000))
    v_true = np.exp(_curve_wt(xs, A, B, scale, mge_coef, bh_coef))
    v_dev = _emulate_device(xs, exp_bias, c1, ts_, tbp, ta_, hs1, hs2p, hismax)
    emu_err = float(np.abs(v_dev / v_true - 1.0).max())
    rep["emu_err"] = emu_err

    cf = np.zeros(NCOEF, dtype=np.float32)
    cf[1] = exp_bias - np.log(OUT_SCALE)  # Exp bias; device emits v/OUT_SCALE
    cf[2] = c1
    for k in range(NT):
        cf[4 + 2 * k] = ts_[k]
        cf[5 + 2 * k] = tbp[k]
    off = 4 + 2 * NT
    for j in range(NH):
        cf[off + 2 * j] = hs1[j]
        cf[off + 2 * j + 1] = hs2p[j]

    # weight matrix: NTERMS diagonal 128x128 f16 blocks: [c1 (linear term,
    # applied straight to the m tile), hinge tiles (pre-signed), tanh amps]
    wm = np.zeros((128, 128 * NTERMS), dtype=np.float16)
    eye = np.arange(128)
    wm[eye, eye] = np.float16(c1)
    for j in range(NHT):
        wm[eye, (1 + j) * 128 + eye] = np.float16(1.0)
    for k in range(NT):
        wm[eye, (1 + NHT + k) * 128 + eye] = np.float16(ta_[k])

    res = (cf, wm, tuple(bool(b) for b in hismax), rep)
    _FIT_CACHE[key] = res
    return res


# ---------------------------------------------------------------------------
# Bass kernel
# ---------------------------------------------------------------------------

_NC_CACHE = {}


def _build_nc(hismax=(True,) * NH):
    key = (FREE, CH, NT, NH, tuple(hismax))
    if key in _NC_CACHE:
        return _NC_CACHE[key]
    import concourse.bass as bass
    import concourse.bacc as bacc
    import concourse.mybir as mybir
    from concourse.tile import TileContext

    F = mybir.ActivationFunctionType
    ALU = mybir.AluOpType
    f32 = mybir.dt.float32
    f16 = mybir.dt.float16
    bf16 = mybir.dt.bfloat16
    i16 = mybir.dt.int16

    nc = bacc.Bacc("TRN2", target_bir_lowering=False, debug=False)
    x_d = nc.dram_tensor("x", [128, FREE], bf16, kind="ExternalInput")
    cf_d = nc.dram_tensor("cf", [NCOEF], f32, kind="ExternalInput")
    w_d = nc.dram_tensor("wm", [128, 128 * NTERMS], f16, kind="ExternalInput")
    out_d = nc.dram_tensor("out", [128, FREE], f16, kind="ExternalOutput")

    HOFF = 4 + 2 * NT

    with TileContext(nc) as tc:
        with (
            tc.tile_pool(name="singles", bufs=1) as singles,
            tc.tile_pool(name="resident", bufs=1) as resident,
            tc.tile_pool(name="xpool", bufs=3) as xpool,
            tc.tile_pool(name="phip", bufs=2) as phip,
            tc.tile_pool(name="tp0", bufs=2) as tp0,
            tc.tile_pool(name="tp1", bufs=2) as tp1,
            tc.tile_pool(name="tp2", bufs=2) as tp2,
            tc.tile_pool(name="outp", bufs=2) as outp,
            tc.tile_pool(name="psum", bufs=4, space="PSUM") as psum,
        ):
            # dummy activation so the single (exp_and_others) table load is
            # inserted up front and overlaps the coefficient DMAs
            dummy = singles.tile([128, 8], f32)
            nc.vector.memset(dummy[:], 1.0)
            nc.scalar.activation(dummy[:], dummy[:], F.Tanh, scale=1.0)

            # coefficient row broadcast to all 128 partitions; weights
            # straight.  The DMAs are issued AFTER the first x slices (the
            # fast-log start is cf-free); consumers pick up cf/wm
            # dependencies individually — no barrier.
            cf = singles.tile([128, NCOEF], f32)
            wm = singles.tile([128, 128 * NTERMS], f16)

            def emit_coef_dmas():
                cf_ap = cf_d[:]
                cf_bcast = bass.AP(
                    tensor=cf_ap.tensor, offset=cf_ap.offset,
                    ap=[[0, 128]] + list(cf_ap.ap),
                )
                nc.sync.dma_start(out=cf[:], in_=cf_bcast)
                nc.sync.dma_start(out=wm[:], in_=w_d[:])

            mres = resident.tile([128, FREE], f16)

            def SL(c):
                return slice(c * CH, (c + 1) * CH)

            # fast-log (no ACT involvement at all); emitted per FL_CH slice,
            # interleaved into the chunk loop below for pipelining:
            # m_raw = ln2/128 * i + (-127*ln2 - mid)   [DVE ts, i = bf16 bits]
            # s     = (i & 127) / 128                  [DVE ts]
            # w     = alpha * (1 - s)                  [DVE ts]
            # s    *= w                                [GPS tt, in place]
            # mres  = m_raw + s                        [GPS tt]
            def emit_fastlog(fc):
                fsl = slice(fc * FL_CH, (fc + 1) * FL_CH)
                xin = xpool.tile([128, FL_CH], bf16, tag="xin")
                nc.sync.dma_start(out=xin[:], in_=x_d[:, fsl])
                iview = xin[:].bitcast(i16)
                mraw = xpool.tile([128, FL_CH], f16, tag="mraw")
                nc.vector.tensor_scalar(
                    out=mraw[:], in0=iview, scalar1=float(LN2 / 128.0),
                    scalar2=float(-127.0 * LN2), op0=ALU.mult, op1=ALU.add,
                )
                mi = xpool.tile([128, FL_CH], i16, tag="mi")
                nc.vector.tensor_scalar(
                    out=mi[:], in0=iview, scalar1=int(127), scalar2=None,
                    op0=ALU.bitwise_and,
                )
                sfr = xpool.tile([128, FL_CH], f16, tag="sfr")
                nc.vector.tensor_scalar(
                    out=sfr[:], in0=mi[:], scalar1=FL_C1,
                    scalar2=float(0.0), op0=ALU.mult, op1=ALU.add,
                )
                wfl = xpool.tile([128, FL_CH], f16, tag="wfl")
                nc.vector.tensor_scalar(
                    out=wfl[:], in0=sfr[:], scalar1=float(-1.0),
                    scalar2=float(128.0 * FL_C1), op0=ALU.mult, op1=ALU.add,
                )
                nc.gpsimd.tensor_tensor(
                    out=sfr[:], in0=sfr[:], in1=wfl[:], op=ALU.mult
                )
                nc.gpsimd.tensor_tensor(
                    out=mres[:, fsl], in0=mraw[:], in1=sfr[:], op=ALU.add
                )

            FL_PER_CH = CH // FL_CH
            # prime the pipeline with the first two fast-log slices
            emit_fastlog(0)
            emit_fastlog(1)
            emit_coef_dmas()

            # basis + PE accumulation + Exp  (exp_and_others set).
            # PSUM is cut into SC-wide sub-chunks (2 banks each, 4 in
            # flight); Exp is emitted 2+ sub-chunks late so the ACT queue
            # never stalls waiting for the PE.
            pending = []

            def emit_exp():
                obase, owidth, acc = pending.pop(0)
                ot = outp.tile([128, SC], f16, tag="ot")
                nc.scalar.activation(
                    ot[:, :owidth], acc[:, :owidth], F.Exp, bias=cf[:, 1:2]
                )
                nc.sync.dma_start(
                    out=out_d[:, obase : obase + owidth], in_=ot[:, :owidth]
                )

            # variable-width big chunks: the first CH is laddered
            # (1024+1024+2048) so the first tanh starts right after the
            # first fast-log slice instead of waiting for all of chunk 0
            chunk_list = [(0, 1024), (1024, 3072)]
            off = CH
            while off < FREE:
                chunk_list.append((off, CH))
                off += CH
            fl_emitted = 2          # priming above covered [0, 2*FL_CH)

            tpools = [tp0, tp1, tp2]
            for ci, (base, width) in enumerate(chunk_list):
                sl = slice(base, base + width)
                # fast-log for the NEXT big chunk streams while this one
                # computes
                if ci + 1 < len(chunk_list):
                    nb, nw = chunk_list[ci + 1]
                    need = (nb + nw) // FL_CH
                    while fl_emitted < need:
                        emit_fastlog(fl_emitted)
                        fl_emitted += 1

                hj_tiles = []
                for j in range(NH):
                    hj = phip.tile([128, CH], f16, tag=f"h{j}")
                    nc.vector.tensor_scalar(
                        out=hj[:, :width], in0=mres[:, sl],
                        scalar1=cf[:, HOFF + 2 * j : HOFF + 2 * j + 1],
                        scalar2=cf[:, HOFF + 2 * j + 1 : HOFF + 2 * j + 2],
                        op0=ALU.mult,
                        op1=ALU.max if hismax[j] else ALU.min,
                    )
                    hj_tiles.append(hj)

                # pre-sum hinge pairs (in place) so the PE has 2 fewer terms;
                # one pair on DVE, one on the otherwise-idle GPSIMD
                nc.vector.tensor_tensor(
                    out=hj_tiles[0][:, :width], in0=hj_tiles[0][:, :width],
                    in1=hj_tiles[1][:, :width], op=ALU.add,
                )
                nc.gpsimd.tensor_tensor(
                    out=hj_tiles[2][:, :width], in0=hj_tiles[2][:, :width],
                    in1=hj_tiles[3][:, :width], op=ALU.add,
                )

                # last chunk: fold all hinge tiles into one on the (by
                # then idle) DVE so the PE tail has 2 fewer terms
                last_chunk = ci == len(chunk_list) - 1
                if last_chunk:
                    nc.vector.tensor_tensor(
                        out=hj_tiles[0][:, :width], in0=hj_tiles[0][:, :width],
                        in1=hj_tiles[2][:, :width], op=ALU.add,
                    )
                    nc.vector.tensor_tensor(
                        out=hj_tiles[0][:, :width], in0=hj_tiles[0][:, :width],
                        in1=hj_tiles[4][:, :width], op=ALU.add,
                    )
                    terms = [None, hj_tiles[0]]
                else:
                    terms = [None, hj_tiles[0], hj_tiles[2], hj_tiles[4]]

                ttiles = []
                for k in range(NT):
                    tk = tpools[k].tile([128, CH], f16, tag=f"t{k}")
                    nc.scalar.activation(
                        tk[:, :width], mres[:, sl], F.Tanh,
                        bias=cf[:, 5 + 2 * k : 6 + 2 * k],
                        scale=cf[:, 4 + 2 * k : 5 + 2 * k],
                    )
                    ttiles.append(tk)

                # staged PSUM accumulation per sub-chunk: first the early
                # group (linear + hinges, ready as soon as DVE runs), then
                # the tanh groups per sub-chunk with the Exp fired as each
                # closes, so the ACT queue and PSUM banks never back up.
                nsub = width // SC
                accs_c = []
                for s in range(nsub):
                    acc = psum.tile([128, SC], f32, tag="acc")
                    accs_c.append(acc)
                    for q in range(SC // MM):
                        qs = slice(s * SC + q * MM, s * SC + (q + 1) * MM)
                        aqs = slice(q * MM, (q + 1) * MM)
                        for i in range(len(terms)):
                            if i == 0:
                                tl = mres[:, base + s * SC + q * MM :
                                          base + s * SC + (q + 1) * MM]
                            else:
                                tl = terms[i][:, qs]
                            nc.tensor.matmul(
                                acc[:, aqs],
                                wm[:, i * 128 : (i + 1) * 128],
                                tl,
                                start=(i == 0),
                                stop=False,
                            )
                for s in range(nsub):
                    acc = accs_c[s]
                    for k in range(NT):
                        wslot = slice((1 + NHT + k) * 128,
                                      (1 + NHT + k + 1) * 128)
                        last = k == NT - 1
                        for q in range(SC // MM):
                            qs = slice(s * SC + q * MM, s * SC + (q + 1) * MM)
                            aqs = slice(q * MM, (q + 1) * MM)
                            nc.tensor.matmul(
                                acc[:, aqs],
                                wm[:, wslot],
                                ttiles[k][:, qs],
                                start=False,
                                stop=last,
                            )
                    # Exp as soon as this sub-chunk's accumulation closes,
                    # freeing its PSUM banks promptly
                    pending.append((base + s * SC, SC, acc))
                    emit_exp()

    nc.finalize()
    _NC_CACHE[key] = nc
    return nc


def _make_in_maps(inputs):
    R_map = np.ascontiguousarray(np.asarray(inputs["R_map"], dtype=np.float32))
    surf = np.asarray(inputs["surf"], dtype=np.float64)
    sigma = np.asarray(inputs["sigma"], dtype=np.float64)
    qintr = np.asarray(inputs["qintr"], dtype=np.float64)
    M_to_L = float(np.asarray(inputs["M_to_L"]))
    inc = float(np.asarray(inputs["inc"]))
    m_bh = float(np.asarray(inputs["m_bh"]))

    r_max = float(R_map.max())
    cf, wm, hismax, rep = _coef_arrays(
        surf, sigma, qintr, M_to_L, inc, m_bh, r_max
    )

    in_maps = []
    for c in range(N_CORES):
        import ml_dtypes

        shard = R_map[c * ROWS_PER_CORE : (c + 1) * ROWS_PER_CORE, :].reshape(128, FREE)
        shard16 = np.ascontiguousarray(shard.astype(ml_dtypes.bfloat16))
        in_maps.append({"x": shard16, "cf": cf, "wm": wm})
    return in_maps, hismax


def _prepare(inputs):
    in_maps, hismax = _make_in_maps(inputs)
    nc = _build_nc(hismax)
    return nc, in_maps


def kernel(**inputs):
    from concourse.bass_utils import run_bass_kernel_spmd

    nc, in_maps = _prepare(inputs)

    res = run_bass_kernel_spmd(nc, in_maps, core_ids=list(range(N_CORES)))
    out = np.empty((ROWS, COLS), dtype=np.float32)
    for c in range(N_CORES):
        shard = res.results[c]["out"].astype(np.float32) * np.float32(OUT_SCALE)
        out[c * ROWS_PER_CORE : (c + 1) * ROWS_PER_CORE, :] = shard.reshape(
            ROWS_PER_CORE, COLS
        )
    return out


if __name__ == "__main__":
    # smoke test of the host fit alone
    rng = np.random.RandomState(0)
    surf = rng.uniform(10, 1010, 16)
    sigma = rng.uniform(5, 205, 16)
    qintr = rng.uniform(0.3, 0.9, 16)
    cf, wm, rep = _coef_arrays(surf, sigma, qintr, 2.0, 1.0, 8.0, 5000.01)
    print("fit report:", rep)
